# revision 65
# baseline (speedup 1.0000x reference)
"""Multi-head attention (softmax+1) for TRN2, 8 NeuronCores.

Sharding: data-parallel over batch B=2 (4 cores per batch) x tensor-parallel
over the 16 heads (4 heads per core).  Each core computes its 4 heads'
QKV projections, attention, and a partial output projection; the host sums
the 4 partials per batch and adds the output bias.

Per-core kernel (S=2048, DM=1024, HD=64, Hloc=4):
  QT[d,q] / KT[d,k] head-transposed layouts from x^T inputs (PE matmuls),
  V'[k, 4*65] natural layout with a ones column per head (denominator trick),
  scores^T[k,q] -> exp on ACT (scale folded into Wq) -> U^T = V'^T @ expT
  (row 64 of each head's block = softmax denominator), normalization via
  1/(1+den) broadcast, partial out-projection.

Schedule notes (this revision):
  - Projection bias-adds are emitted immediately after each PSUM
    accumulation stops and alternate DVE/ACT, so qt/kt column blocks
    release early and K's m-loop is not paced by a single engine.
  - PE warmup: ~14 dummy matmuls + LDWEIGHTS gap-fillers keep the HAM
    clock gate at 8/8 through the DMA-gated projection start; a tiny exp
    preloads the ACT table and a dummy partition_broadcast preloads the
    GpSimd Q7 library (otherwise a ~7us swap hits the first boundary).
  - Quarters run qq-major ((0,0),(1,0),(0,1),(1,1),...) so each q-block's
    out-projection becomes available two quarters later and drains evenly
    (1 half per group from group 8) without head-of-line blocking the PE
    behind the previous quarter's normalize muls.
  - The attention phase is ACT(exp)-bound (~1.01us per [128,1024] exp)
    outside the first quarter; the PE stream (scores pair + V-accum +
    interleaved vproj/outproj) fits underneath.
  - Tail: only the last q-block's 8 out-proj halves remain after the last
    exp.  Its normalize splits across ACT (den-adds, r16 casts) and DVE
    (u-copies, recips) with a PE ones-matmul broadcast; the dead scores/U
    PSUM pools are recycled into a wide tail pool and the final casts
    alternate scalar/vector engines.
"""

import contextlib
import sys

if "/opt/trn_rl_repo" not in sys.path:
    sys.path.insert(0, "/opt/trn_rl_repo")

import numpy as np

import concourse.bass as bass
import concourse.mybir as mybir
import concourse.tile as tile
from concourse import bacc
from concourse.bass_utils import run_bass_kernel_spmd

F32 = mybir.dt.float32
F32R = mybir.dt.float32r
F16 = mybir.dt.float16
EXP = mybir.ActivationFunctionType.Exp
IDENT = mybir.ActivationFunctionType.Identity

B, S, DM = 2, 2048, 1024
H, HD = 16, 64
SCALE = HD ** -0.5
HLOC = 4              # heads per core
CD = HLOC * HD        # 256 local head dims
VW = HD + 1           # 65: V columns + ones column per head
MC = DM // 128        # 8 contraction chunks for projections
KT16 = S // 128       # 16 sequence tiles
W260 = HLOC * VW      # 260

_CACHE = {}
LAST_RESULT = None


def _build():
    nc = bacc.Bacc()
    dp = nc.declare_dram_parameter
    xq_d = dp("xq", [DM, S], F16, isOutput=False)    # query[b]^T
    xk_d = dp("xk", [DM, S], F16, isOutput=False)
    xv_d = dp("xv", [DM, S], F16, isOutput=False)
    wq_d = dp("wq", [DM, CD], F16, isOutput=False)   # (SCALE * Wq_shard)^T
    wk_d = dp("wk", [DM, CD], F16, isOutput=False)   # Wk_shard^T
    wv_d = dp("wv", [DM, W260], F16, isOutput=False)  # Wv^T 260-layout, zeros in ones-cols
    wo_d = dp("wo", [CD, DM], F16, isOutput=False)   # Wo_shard^T
    bq_d = dp("bq", [128, 2], F32, isOutput=False)   # bias cols per 128-pair (SCALE-folded)
    bk_d = dp("bk", [128, 2], F32, isOutput=False)
    bv_d = dp("bv", [1, W260], F16, isOutput=False)  # [bv_h | 1.0] blocks
    on_d = dp("ones1", [1, 128], F16, isOutput=False)
    out_d = dp("out", [S, DM], F16, isOutput=True)   # partial (pre-bo) projection

    with tile.TileContext(nc) as tc:
        with tc.tile_pool(name="weights", bufs=1) as wpool, \
             tc.tile_pool(name="persist", bufs=1) as perst:
            wq_sb = wpool.tile([128, MC, CD], F16)
            wk_sb = wpool.tile([128, MC, CD], F16)
            wv_sb = wpool.tile([128, MC, W260], F16)
            wo_sb = wpool.tile([128, 2, DM], F16)
            bq_sb = wpool.tile([128, 2], F32)
            bk_sb = wpool.tile([128, 2], F32)
            bv_sb = wpool.tile([1, W260], F16)
            on_sb = wpool.tile([1, 128], F16)

            qt_sb = perst.tile([128, 2, S], F16)   # [d(2 heads), pair, q]
            kt_sb = perst.tile([128, 2, S], F16)
            v_sb = perst.tile([128, KT16, W260], F16)  # [k, ktile, 4*(V|1)]
            at_sb = perst.tile([128, 2, S], F16)   # normalized attn out^T
            xv_sb = perst.tile([128, MC, S], F16)  # resident value^T chunks
            warm_sb = wpool.tile([128, 512], F16)  # zero tile for PE warmup
            wout_sb = wpool.tile([1, 128], F16)    # scratch for ACT table preload
            ones32 = wpool.tile([1, 64], F32)      # f32 ones row for PE broadcast

            # ------------- Phase 1: Q and K projections ----------------
            # Bias-adds are emitted right after each j-block's accumulation
            # stops and round-robin across DVE/ACT/GpSimd so qt/kt release
            # quickly and the K projection isn't paced by a single engine.
            with tc.tile_pool(name="xs", bufs=16) as xs, \
                 tc.tile_pool(name="pproj", bufs=8, space="PSUM") as pproj:
                # PE warmup: ~20 dummy matmuls on a memset tile lift the HAM
                # clock gate to 8/8 before the real projections start, and a
                # tiny exp preloads the ACT function table off-critical-path.
                # A dummy partition_broadcast makes GpSimd load its Q7 library
                # now instead of at the first quarter boundary (~7us swap).
                nc.vector.memset(warm_sb[:], 0.0)
                nc.vector.memset(ones32[:], 1.0)
                nc.scalar.activation(out=wout_sb[:], in_=warm_sb[0:1, 0:128],
                                     func=EXP)
                # dummy broadcast so GpSimd loads its Q7 library now instead
                # of at the first quarter boundary (~7us swap)
                pbwarm = wpool.tile([2, 64], F32)
                nc.gpsimd.partition_broadcast(pbwarm[:], ones32[:])
                wps = pproj.tile([128, 256], F32, tag="ps", name="warmps")
                for _ in range(14):
                    nc.tensor.matmul(wps[:], warm_sb[:, 0:128], warm_sb[:, 0:256],
                                     start=True, stop=True)

                def pe_keepwarm(n):
                    # dependency-free LDWEIGHTS that fill PE idle slots during
                    # DMA-gated stretches so HAM never drops to 4/8.  No PSUM
                    # write, and every real matmul self-loads its own weights,
                    # so clobbering the weight buffer is harmless.
                    for _ in range(n):
                        nc.tensor.ldweights(warm_sb[:, 0:128])
                nc.sync.dma_start(
                    out=wq_sb[:], in_=wq_d.ap().rearrange("(m p) c -> p m c", m=MC))
                nc.sync.dma_start(out=bq_sb[:], in_=bq_d.ap())

                def bias_add(idx, dst_ap, ps_ap, b_ap):
                    # alternate DVE / ACT so neither engine paces the release
                    # (GpSimd cannot read PSUM)
                    if idx % 2 == 0:
                        nc.vector.tensor_scalar_add(dst_ap, ps_ap, b_ap)
                    else:
                        nc.scalar.activation(out=dst_ap, in_=ps_ap, func=IDENT,
                                             bias=b_ap)

                for src_d, w_sb, b_sb, dst in (
                    (xq_d, wq_sb, bq_sb, qt_sb),
                    (xk_d, wk_sb, bk_sb, kt_sb),
                ):
                    pss = [pproj.tile([128, 512], F32, tag="ps", name=f"ps{k}")
                           for k in range(8)]
                    xts = []
                    for m in range(MC):
                        xt = xs.tile([128, S], F16, tag="xs", name=f"xt{m}")
                        nc.sync.dma_start(out=xt[:], in_=src_d.ap()[m * 128:(m + 1) * 128, :])
                        xts.append(xt)
                    if dst is qt_sb:
                        # K weights enqueue after the xq chunks so xq streams first
                        nc.sync.dma_start(
                            out=wk_sb[:],
                            in_=wk_d.ap().rearrange("(m p) c -> p m c", m=MC))
                        nc.sync.dma_start(out=bk_sb[:], in_=bk_d.ap())
                    for m in range(MC):
                        xt = xts[m]
                        st, sp = (m == 0), (m == MC - 1)
                        for p in range(2):
                            for j in range(4):
                                nc.tensor.matmul(
                                    pss[p * 4 + j][:],
                                    w_sb[:, m, p * 128:(p + 1) * 128],
                                    xt[:, j * 512:(j + 1) * 512],
                                    start=st, stop=sp,
                                )
                                if sp:
                                    # release this 512-col block immediately
                                    bias_add(
                                        p * 4 + j,
                                        dst[:, p, j * 512:(j + 1) * 512],
                                        pss[p * 4 + j][:], b_sb[:, p:p + 1],
                                    )
                        if m < 5:
                            pe_keepwarm(6)
                # stage V weights/input + wo for the attention phase.
                # xv is sent in column quarters so vproj of early k-tiles
                # unblocks after 1MB instead of the full 4MB transfer.
                nc.sync.dma_start(
                    out=wv_sb[:], in_=wv_d.ap().rearrange("(m p) c -> p m c", m=MC))
                nc.sync.dma_start(out=bv_sb[:], in_=bv_d.ap())
                nc.sync.dma_start(out=on_sb[:], in_=on_d.ap())
                for q4 in range(4):
                    c0, c1 = q4 * 512, (q4 + 1) * 512
                    for m in range(MC):
                        nc.sync.dma_start(out=xv_sb[:, m, c0:c1],
                                          in_=xv_d.ap()[m * 128:(m + 1) * 128, c0:c1])
                    if q4 == 1:
                        nc.sync.dma_start(
                            out=wo_sb[:],
                            in_=wo_d.ap().rearrange("(k p) c -> p k c", k=2))

            # ------------- Phase 2: attention, software-pipelined -----------
            with tc.tile_pool(name="expp", bufs=8) as expp, \
                 tc.tile_pool(name="obuf", bufs=4) as obuf, \
                 tc.tile_pool(name="npool", bufs=3) as npool:
                psc_ctx = tc.tile_pool(name="psc", bufs=2, space="PSUM")
                psc = psc_ctx.__enter__()
                put_ctx = tc.tile_pool(name="put", bufs=2, space="PSUM")
                put = put_ctx.__enter__()

                pout = None
                pv_ctx = tc.tile_pool(name="pv", bufs=2, space="PSUM")
                pv = pv_ctx.__enter__()

                def vproj_one(kt):
                    """V projection for one k-tile."""
                    vps = pv.tile([128, W260], F32, tag="vps", name="vps")
                    nc.tensor.matmul(vps[:], on_sb[:], bv_sb[:], start=True, stop=False)
                    for m in range(MC):
                        nc.tensor.matmul(
                            vps[:],
                            xv_sb[:, m, kt * 128:(kt + 1) * 128],
                            wv_sb[:, m, :],
                            start=False, stop=(m == MC - 1),
                        )
                    nc.vector.tensor_copy(v_sb[:, kt, :], vps[:])

                ob_tiles = {}

                def outproj_half(t, n, act_copy=False, pin=True):
                    # Pin the model-time so the scheduler doesn't hoist these
                    # ahead of the normalize muls that produce `at` (its DVE/
                    # GpSimd timing model is optimistic and the PE stream is
                    # in-order, so a hoisted LDWEIGHTS head-of-line blocks).
                    # stagger the pins ~1.2us apart so the 8 halves of a
                    # q-block spread across the drain quarter instead of
                    # bunching right after a shared pin time (which made the
                    # drain groups run ~290ns over the exp budget for 8
                    # consecutive groups)
                    ctx = (tc.tile_wait_until(0.090 + 0.028 * (t // 4)
                                              + 0.0012 * (2 * (t % 4) + n))
                           if pin else contextlib.nullcontext())
                    with ctx:
                        ob = ob_tiles.get(t)
                        if ob is None:
                            ob = obuf.tile([128, DM], F16, tag="ob", name="ob")
                            ob_tiles[t] = ob
                        op = pout.tile([128, 512], F32, tag="op", name="op")
                        for cc in range(2):
                            nc.tensor.matmul(
                                op[:],
                                at_sb[:, cc, t * 128:(t + 1) * 128],
                                wo_sb[:, cc, n * 512:(n + 1) * 512],
                                start=(cc == 0), stop=(cc == 1),
                            )
                        if act_copy:
                            nc.scalar.copy(ob[:, n * 512:(n + 1) * 512], op[:])
                        else:
                            nc.vector.tensor_copy(ob[:, n * 512:(n + 1) * 512], op[:])
                        if n == 1:
                            nc.sync.dma_start(
                                out=out_d.ap()[t * 128:(t + 1) * 128, :], in_=ob[:])
                            del ob_tiles[t]

                def normalize(uts, p, q0, tail=False):
                    # The U accumulator (PSUM) is freed by the den-add (row
                    # 64) + u-copy (rows 0:64); the 1/(1+den) broadcast is a
                    # float32r ones-matmul on the PE (~0.3us, vs ~1us+queue on
                    # GpSimd), so `at` is ready ~2.5us after the quarter ends
                    # and the hoisted out-projection never stalls on it.
                    dens, us = [], []
                    for hh in range(2):
                        den1 = npool.tile([1, 512], F32, tag="den", name=f"den{hh}")
                        if tail:
                            # ACT is idle in the tail: run the den-adds there
                            # so DVE can start the u-copies in parallel.
                            nc.scalar.activation(out=den1[:], in_=uts[hh][64:65, :],
                                                 func=IDENT, bias=1.0)
                        else:
                            nc.vector.tensor_scalar_add(den1[:], uts[hh][64:65, :], 1.0)
                        dens.append(den1)
                        u = npool.tile([64, 512], F32, tag="u", name=f"u{hh}")
                        nc.vector.tensor_copy(u[:], uts[hh][0:64, :])
                        us.append(u)
                    for hh in range(2):
                        po = 64 * hh
                        r = npool.tile([1, 512], F32, tag="r")
                        nc.vector.reciprocal_approx_fast(r[:], dens[hh][:])
                        if tail:
                            # PE ones-matmul broadcast into a free PSUM slot;
                            # shortest-latency path for the final q-block.
                            r16 = npool.tile([1, 512], F16, tag="r16")
                            nc.scalar.copy(r16[:], r[:])
                            rb_ps = pout.tile([64, 512], F32, tag="op", name="rbps")
                            nc.tensor.matmul(rb_ps[:], on_sb[:, 0:64], r16[:],
                                             start=True, stop=True)
                            nc.vector.tensor_mul(
                                at_sb[po:po + 64, p, q0:q0 + 512],
                                us[hh][:], rb_ps[:])
                        else:
                            rb = npool.tile([64, 512], F32, tag="rb")
                            nc.gpsimd.partition_broadcast(rb[:], r[:])
                            nc.vector.tensor_mul(
                                at_sb[po:po + 64, p, q0:q0 + 512], us[hh][:], rb[:])

                # qq-major quarter order: a q-block's two pairs complete in
                # consecutive quarters, so its out-projection spreads over the
                # following quarters instead of bunching at the end.
                QUARTERS = [(p, qq) for qq in range(4) for p in range(2)]
                sched = [(p, qq, i) for (p, qq) in QUARTERS for i in range(KT16)]
                quarters = {}
                hist = []   # per group: [p, qq, i, sc, ex]
                outq = []   # pending out-projection halves
                # scores run one group ahead of exp, and the V-accumulation
                # two behind, so the next ACT's input is always ready the
                # moment the previous ACT retires.
                for it in range(len(sched) + 2):
                    if it < len(sched):
                        p, qq, i = sched[it]
                        q0 = qq * 512
                        sc = psc.tile([128, 1024], F32, tag="sc")
                        for hh in range(2):
                            nc.tensor.matmul(
                                sc[:, hh * 512:(hh + 1) * 512],
                                kt_sb[64 * hh:64 * hh + 64, p, i * 128:(i + 1) * 128],
                                qt_sb[64 * hh:64 * hh + 64, p, q0:q0 + 512],
                                start=True, stop=True,
                            )
                        hist.append([p, qq, i, sc, None])
                    if 1 <= it <= len(sched):
                        e = hist[it - 1]
                        ex = expp.tile([128, 1024], F16, tag="ex")
                        nc.scalar.activation(out=ex[:], in_=e[3][:], func=EXP)
                        e[4] = ex
                        if it == 1:
                            vproj_one(0)   # k-tiles 0,1 behind the first exp
                            vproj_one(1)
                    if it >= 2:
                        g = it - 2
                        pp, pqq, pi, _, pex = hist[g]
                        qi = g // KT16
                        if pi == 0:
                            quarters[(pp, pqq)] = (
                                put.tile([65, 512], F32, tag="ut", name="ut0"),
                                put.tile([65, 512], F32, tag="ut", name="ut1"),
                            )
                        uts = quarters[(pp, pqq)]
                        for hh in range(2):
                            h = 2 * pp + hh
                            nc.tensor.matmul(
                                uts[hh][:],
                                v_sb[:, pi, h * VW:(h + 1) * VW],
                                pex[:, hh * 512:(hh + 1) * 512],
                                start=(pi == 0), stop=(pi == KT16 - 1),
                            )
                        hist[g][4] = None
                        # interleaved extras: vproj stays 2 tiles ahead in the
                        # first quarter; out-proj drains 1 half per group but
                        # only from group 4 on, so the PE never head-of-line
                        # blocks on the previous quarter's normalize muls.
                        if qi == 0:
                            if pi < KT16 - 2:
                                vproj_one(pi + 2)
                        elif outq and pi >= 8:
                            outproj_half(*outq.pop(0))
                        if pi == KT16 - 1:
                            normalize(uts, pp, pqq * 512,
                                      tail=(g == len(sched) - 1))
                            del quarters[(pp, pqq)]
                            if qi == 0:
                                # pool swap: pv's 2 banks + one of put's go to
                                # a 3-slot ut pool, so later quarter-boundary
                                # ut handoffs never stall the V-accumulation
                                # (the den-add+u-copy release lags ~1.5-2us);
                                # the out-proj drain runs on a single slot,
                                # which still fits 8 halves per quarter.
                                pv_ctx.__exit__(None, None, None)
                                put_ctx.__exit__(None, None, None)
                                put_ctx = tc.tile_pool(name="put2", bufs=3,
                                                       space="PSUM")
                                put = put_ctx.__enter__()
                                pout_ctx = tc.tile_pool(name="pout", bufs=1,
                                                        space="PSUM")
                                pout = pout_ctx.__enter__()
                            if pp == 1:
                                # q-block pqq fully normalized
                                outq.extend([(pqq * 4 + tt, n)
                                             for tt in range(4) for n in range(2)])
                # final q-block's out-projection (ACT is idle by now);
                # alternate the PSUM->SBUF casts between scalar and vector.
                # The scores/U pools are dead now — recycle their banks into
                # a wide tail pool so all 8 halves pipeline without waiting
                # on cast->slot recycling.  A few LDWEIGHTS keep the PE clock
                # warm through the normalize-chain idle gap.
                pout_ctx.__exit__(None, None, None)
                put_ctx.__exit__(None, None, None)
                psc_ctx.__exit__(None, None, None)
                ptail_ctx = tc.tile_pool(name="ptail", bufs=6, space="PSUM")
                pout = ptail_ctx.__enter__()
                for _ in range(36):
                    nc.tensor.ldweights(warm_sb[:, 0:128])
                for idx, (t, n) in enumerate(outq):
                    outproj_half(t, n, act_copy=(idx % 2 == 0))
                ptail_ctx.__exit__(None, None, None)

    nc.finalize()
    return nc


def kernel(query, key, value, Wq, bq, Wk, bk, Wv, bv, Wo, bo):
    global LAST_RESULT
    if "nc" not in _CACHE:
        _CACHE["nc"] = _build()
    nc = _CACHE["nc"]

    query = np.asarray(query, np.float32)
    key = np.asarray(key, np.float32)
    value = np.asarray(value, np.float32)
    Wq = np.asarray(Wq, np.float32)
    Wk = np.asarray(Wk, np.float32)
    Wv = np.asarray(Wv, np.float32)
    Wo = np.asarray(Wo, np.float32)
    bq = np.asarray(bq, np.float32)
    bk = np.asarray(bk, np.float32)
    bv = np.asarray(bv, np.float32)
    bo = np.asarray(bo, np.float32)

    xqT = [np.ascontiguousarray(query[b].T).astype(np.float16) for b in range(B)]
    xkT = [np.ascontiguousarray(key[b].T).astype(np.float16) for b in range(B)]
    xvT = [np.ascontiguousarray(value[b].T).astype(np.float16) for b in range(B)]

    ones1 = np.ones((1, 128), np.float16)
    in_maps = []
    for c in range(8):
        b, hg = c // 4, c % 4
        r0 = hg * CD
        wq_s = np.ascontiguousarray((Wq[r0:r0 + CD, :] * SCALE).T).astype(np.float16)
        wk_s = np.ascontiguousarray(Wk[r0:r0 + CD, :].T).astype(np.float16)
        wo_s = np.ascontiguousarray(Wo[:, r0:r0 + CD].T).astype(np.float16)
        bq_s = np.ascontiguousarray((bq[r0:r0 + CD] * SCALE).reshape(2, 128).T)  # [128,2]
        bk_s = np.ascontiguousarray(bk[r0:r0 + CD].reshape(2, 128).T)
        # V weights/bias in 260-layout: [64 cols of head | bias-1 col] x4
        wv260 = np.zeros((DM, W260), np.float32)
        bv260 = np.zeros((1, W260), np.float32)
        for hh in range(HLOC):
            wv260[:, hh * VW:hh * VW + HD] = Wv[r0 + hh * HD:r0 + (hh + 1) * HD, :].T
            bv260[0, hh * VW:hh * VW + HD] = bv[r0 + hh * HD:r0 + (hh + 1) * HD]
            bv260[0, hh * VW + HD] = 1.0
        in_maps.append({
            "xq": xqT[b], "xk": xkT[b], "xv": xvT[b],
            "wq": wq_s, "wk": wk_s, "wv": np.ascontiguousarray(wv260).astype(np.float16),
            "wo": wo_s, "bq": bq_s, "bk": bk_s, "bv": bv260.astype(np.float16),
            "ones1": ones1,
        })

    res = run_bass_kernel_spmd(nc, in_maps, core_ids=list(range(8)))
    LAST_RESULT = res

    out = np.empty((B, S, DM), np.float32)
    for b in range(B):
        acc = np.zeros((S, DM), np.float64)
        for hg in range(4):
            acc += res.results[b * 4 + hg]["out"].astype(np.float64)
        out[b] = (acc + bo.astype(np.float64)).astype(np.float32)
    return out


# revision 66
# speedup vs baseline: 1.0045x; 1.0045x over previous
"""Multi-head attention (softmax+1) for TRN2, 8 NeuronCores.

Sharding: data-parallel over batch B=2 (4 cores per batch) x tensor-parallel
over the 16 heads (4 heads per core).  Each core computes its 4 heads'
QKV projections, attention, and a partial output projection; the host sums
the 4 partials per batch and adds the output bias.

Per-core kernel (S=2048, DM=1024, HD=64, Hloc=4):
  QT[d,q] / KT[d,k] head-transposed layouts from x^T inputs (PE matmuls),
  V'[k, 4*65] natural layout with a ones column per head (denominator trick),
  scores^T[k,q] -> exp on ACT (scale folded into Wq) -> U^T = V'^T @ expT
  (row 64 of each head's block = softmax denominator), normalization via
  1/(1+den) broadcast, partial out-projection.

Schedule notes (this revision):
  - Projection bias-adds are emitted immediately after each PSUM
    accumulation stops and alternate DVE/ACT, so qt/kt column blocks
    release early and K's m-loop is not paced by a single engine.
  - PE warmup: ~14 dummy matmuls + LDWEIGHTS gap-fillers keep the HAM
    clock gate at 8/8 through the DMA-gated projection start; a tiny exp
    preloads the ACT table and a dummy partition_broadcast preloads the
    GpSimd Q7 library (otherwise a ~7us swap hits the first boundary).
  - Quarters run qq-major ((0,0),(1,0),(0,1),(1,1),...) so each q-block's
    out-projection becomes available two quarters later and drains evenly
    (1 half per group from group 8) without head-of-line blocking the PE
    behind the previous quarter's normalize muls.
  - The attention phase is ACT(exp)-bound (~1.01us per [128,1024] exp)
    outside the first quarter; the PE stream (scores pair + V-accum +
    interleaved vproj/outproj) fits underneath.
  - Tail: only the last q-block's 8 out-proj halves remain after the last
    exp.  Its normalize splits across ACT (den-adds, r16 casts) and DVE
    (u-copies, recips) with a PE ones-matmul broadcast; the dead scores/U
    PSUM pools are recycled into a wide tail pool and the final casts
    alternate scalar/vector engines.
"""

import contextlib
import sys

if "/opt/trn_rl_repo" not in sys.path:
    sys.path.insert(0, "/opt/trn_rl_repo")

import numpy as np

import concourse.bass as bass
import concourse.mybir as mybir
import concourse.tile as tile
from concourse import bacc
from concourse.bass_utils import run_bass_kernel_spmd

F32 = mybir.dt.float32
F32R = mybir.dt.float32r
F16 = mybir.dt.float16
EXP = mybir.ActivationFunctionType.Exp
IDENT = mybir.ActivationFunctionType.Identity

B, S, DM = 2, 2048, 1024
H, HD = 16, 64
SCALE = HD ** -0.5
HLOC = 4              # heads per core
CD = HLOC * HD        # 256 local head dims
VW = HD + 1           # 65: V columns + ones column per head
MC = DM // 128        # 8 contraction chunks for projections
KT16 = S // 128       # 16 sequence tiles
W260 = HLOC * VW      # 260

_CACHE = {}
LAST_RESULT = None


def _build():
    nc = bacc.Bacc()
    dp = nc.declare_dram_parameter
    xq_d = dp("xq", [DM, S], F16, isOutput=False)    # query[b]^T
    xk_d = dp("xk", [DM, S], F16, isOutput=False)
    xv_d = dp("xv", [DM, S], F16, isOutput=False)
    wq_d = dp("wq", [DM, CD], F16, isOutput=False)   # (SCALE * Wq_shard)^T
    wk_d = dp("wk", [DM, CD], F16, isOutput=False)   # Wk_shard^T
    wv_d = dp("wv", [DM, W260], F16, isOutput=False)  # Wv^T 260-layout, zeros in ones-cols
    wo_d = dp("wo", [CD, DM], F16, isOutput=False)   # Wo_shard^T
    bq_d = dp("bq", [128, 2], F32, isOutput=False)   # bias cols per 128-pair (SCALE-folded)
    bk_d = dp("bk", [128, 2], F32, isOutput=False)
    bv_d = dp("bv", [1, W260], F16, isOutput=False)  # [bv_h | 1.0] blocks
    on_d = dp("ones1", [1, 128], F16, isOutput=False)
    out_d = dp("out", [S, DM], F16, isOutput=True)   # partial (pre-bo) projection

    with tile.TileContext(nc) as tc:
        with tc.tile_pool(name="weights", bufs=1) as wpool, \
             tc.tile_pool(name="persist", bufs=1) as perst:
            wq_sb = wpool.tile([128, MC, CD], F16)
            wk_sb = wpool.tile([128, MC, CD], F16)
            wv_sb = wpool.tile([128, MC, W260], F16)
            wo_sb = wpool.tile([128, 2, DM], F16)
            bq_sb = wpool.tile([128, 2], F32)
            bk_sb = wpool.tile([128, 2], F32)
            bv_sb = wpool.tile([1, W260], F16)
            on_sb = wpool.tile([1, 128], F16)

            qt_sb = perst.tile([128, 2, S], F16)   # [d(2 heads), pair, q]
            kt_sb = perst.tile([128, 2, S], F16)
            v_sb = perst.tile([128, KT16, W260], F16)  # [k, ktile, 4*(V|1)]
            at_sb = perst.tile([128, 2, S], F16)   # normalized attn out^T
            xv_sb = perst.tile([128, MC, S], F16)  # resident value^T chunks
            warm_sb = wpool.tile([128, 512], F16)  # zero tile for PE warmup
            wout_sb = wpool.tile([1, 128], F16)    # scratch for ACT table preload
            ones32 = wpool.tile([1, 64], F32)      # f32 ones row for PE broadcast

            # ------------- Phase 1: Q and K projections ----------------
            # Bias-adds are emitted right after each j-block's accumulation
            # stops and round-robin across DVE/ACT/GpSimd so qt/kt release
            # quickly and the K projection isn't paced by a single engine.
            with tc.tile_pool(name="xs", bufs=16) as xs, \
                 tc.tile_pool(name="pproj", bufs=8, space="PSUM") as pproj:
                # PE warmup: ~20 dummy matmuls on a memset tile lift the HAM
                # clock gate to 8/8 before the real projections start, and a
                # tiny exp preloads the ACT function table off-critical-path.
                # A dummy partition_broadcast makes GpSimd load its Q7 library
                # now instead of at the first quarter boundary (~7us swap).
                nc.vector.memset(warm_sb[:], 0.0)
                nc.vector.memset(ones32[:], 1.0)
                nc.scalar.activation(out=wout_sb[:], in_=warm_sb[0:1, 0:128],
                                     func=EXP)
                # dummy broadcast so GpSimd loads its Q7 library now instead
                # of at the first quarter boundary (~7us swap)
                pbwarm = wpool.tile([2, 64], F32)
                nc.gpsimd.partition_broadcast(pbwarm[:], ones32[:])
                wps = pproj.tile([128, 256], F32, tag="ps", name="warmps")
                for _ in range(14):
                    nc.tensor.matmul(wps[:], warm_sb[:, 0:128], warm_sb[:, 0:256],
                                     start=True, stop=True)

                def pe_keepwarm(n):
                    # dependency-free LDWEIGHTS that fill PE idle slots during
                    # DMA-gated stretches so HAM never drops to 4/8.  No PSUM
                    # write, and every real matmul self-loads its own weights,
                    # so clobbering the weight buffer is harmless.
                    for _ in range(n):
                        nc.tensor.ldweights(warm_sb[:, 0:128])
                nc.sync.dma_start(
                    out=wq_sb[:], in_=wq_d.ap().rearrange("(m p) c -> p m c", m=MC))
                nc.sync.dma_start(out=bq_sb[:], in_=bq_d.ap())

                def bias_add(idx, dst_ap, ps_ap, b_ap):
                    # alternate DVE / ACT so neither engine paces the release
                    # (GpSimd cannot read PSUM)
                    if idx % 2 == 0:
                        nc.vector.tensor_scalar_add(dst_ap, ps_ap, b_ap)
                    else:
                        nc.scalar.activation(out=dst_ap, in_=ps_ap, func=IDENT,
                                             bias=b_ap)

                for src_d, w_sb, b_sb, dst in (
                    (xq_d, wq_sb, bq_sb, qt_sb),
                    (xk_d, wk_sb, bk_sb, kt_sb),
                ):
                    pss = [pproj.tile([128, 512], F32, tag="ps", name=f"ps{k}")
                           for k in range(8)]
                    xts = []
                    for m in range(MC):
                        xt = xs.tile([128, S], F16, tag="xs", name=f"xt{m}")
                        nc.sync.dma_start(out=xt[:], in_=src_d.ap()[m * 128:(m + 1) * 128, :])
                        xts.append(xt)
                    if dst is qt_sb:
                        # K weights enqueue after the xq chunks so xq streams first
                        nc.sync.dma_start(
                            out=wk_sb[:],
                            in_=wk_d.ap().rearrange("(m p) c -> p m c", m=MC))
                        nc.sync.dma_start(out=bk_sb[:], in_=bk_d.ap())
                    for m in range(MC):
                        xt = xts[m]
                        st, sp = (m == 0), (m == MC - 1)
                        for p in range(2):
                            for j in range(4):
                                nc.tensor.matmul(
                                    pss[p * 4 + j][:],
                                    w_sb[:, m, p * 128:(p + 1) * 128],
                                    xt[:, j * 512:(j + 1) * 512],
                                    start=st, stop=sp,
                                )
                                if sp:
                                    # release this 512-col block immediately
                                    bias_add(
                                        p * 4 + j,
                                        dst[:, p, j * 512:(j + 1) * 512],
                                        pss[p * 4 + j][:], b_sb[:, p:p + 1],
                                    )
                        if m < 5:
                            pe_keepwarm(6)
                # stage V weights/input + wo for the attention phase.
                # xv is sent in column quarters so vproj of early k-tiles
                # unblocks after 1MB instead of the full 4MB transfer.
                nc.sync.dma_start(
                    out=wv_sb[:], in_=wv_d.ap().rearrange("(m p) c -> p m c", m=MC))
                nc.sync.dma_start(out=bv_sb[:], in_=bv_d.ap())
                nc.sync.dma_start(out=on_sb[:], in_=on_d.ap())
                for q4 in range(4):
                    c0, c1 = q4 * 512, (q4 + 1) * 512
                    for m in range(MC):
                        nc.sync.dma_start(out=xv_sb[:, m, c0:c1],
                                          in_=xv_d.ap()[m * 128:(m + 1) * 128, c0:c1])
                    if q4 == 1:
                        nc.sync.dma_start(
                            out=wo_sb[:],
                            in_=wo_d.ap().rearrange("(k p) c -> p k c", k=2))

            # ------------- Phase 2: attention, software-pipelined -----------
            with tc.tile_pool(name="expp", bufs=8) as expp, \
                 tc.tile_pool(name="obuf", bufs=4) as obuf, \
                 tc.tile_pool(name="npool", bufs=3) as npool:
                psc_ctx = tc.tile_pool(name="psc", bufs=2, space="PSUM")
                psc = psc_ctx.__enter__()
                put_ctx = tc.tile_pool(name="put", bufs=2, space="PSUM")
                put = put_ctx.__enter__()

                pout = None
                pv_ctx = tc.tile_pool(name="pv", bufs=2, space="PSUM")
                pv = pv_ctx.__enter__()

                def vproj_one(kt):
                    """V projection for one k-tile."""
                    vps = pv.tile([128, W260], F32, tag="vps", name="vps")
                    nc.tensor.matmul(vps[:], on_sb[:], bv_sb[:], start=True, stop=False)
                    for m in range(MC):
                        nc.tensor.matmul(
                            vps[:],
                            xv_sb[:, m, kt * 128:(kt + 1) * 128],
                            wv_sb[:, m, :],
                            start=False, stop=(m == MC - 1),
                        )
                    nc.vector.tensor_copy(v_sb[:, kt, :], vps[:])

                ob_tiles = {}

                def outproj_half(t, n, act_copy=False, pin=True):
                    # Pin the model-time so the scheduler doesn't hoist these
                    # ahead of the normalize muls that produce `at` (its DVE/
                    # GpSimd timing model is optimistic and the PE stream is
                    # in-order, so a hoisted LDWEIGHTS head-of-line blocks).
                    # stagger the pins ~1.2us apart so the 8 halves of a
                    # q-block spread across the drain quarter instead of
                    # bunching right after a shared pin time (which made the
                    # drain groups run ~290ns over the exp budget for 8
                    # consecutive groups)
                    ctx = (tc.tile_wait_until(0.090 + 0.028 * (t // 4)
                                              + 0.0012 * (2 * (t % 4) + n))
                           if pin else contextlib.nullcontext())
                    with ctx:
                        ob = ob_tiles.get(t)
                        if ob is None:
                            ob = obuf.tile([128, DM], F16, tag="ob", name="ob")
                            ob_tiles[t] = ob
                        op = pout.tile([128, 512], F32, tag="op", name="op")
                        for cc in range(2):
                            nc.tensor.matmul(
                                op[:],
                                at_sb[:, cc, t * 128:(t + 1) * 128],
                                wo_sb[:, cc, n * 512:(n + 1) * 512],
                                start=(cc == 0), stop=(cc == 1),
                            )
                        if act_copy:
                            nc.scalar.copy(ob[:, n * 512:(n + 1) * 512], op[:])
                        else:
                            nc.vector.tensor_copy(ob[:, n * 512:(n + 1) * 512], op[:])
                        if n == 1:
                            nc.sync.dma_start(
                                out=out_d.ap()[t * 128:(t + 1) * 128, :], in_=ob[:])
                            del ob_tiles[t]

                def normalize(uts, p, q0, tail=False):
                    # The U accumulator (PSUM) is freed by the den-add (row
                    # 64) + u-copy (rows 0:64); the 1/(1+den) broadcast is a
                    # float32r ones-matmul on the PE (~0.3us, vs ~1us+queue on
                    # GpSimd), so `at` is ready ~2.5us after the quarter ends
                    # and the hoisted out-projection never stalls on it.
                    dens, us = [], []
                    for hh in range(2):
                        den1 = npool.tile([1, 512], F32, tag="den", name=f"den{hh}")
                        if tail:
                            # ACT is idle in the tail: run the den-adds there
                            # so DVE can start the u-copies in parallel.
                            nc.scalar.activation(out=den1[:], in_=uts[hh][64:65, :],
                                                 func=IDENT, bias=1.0)
                        else:
                            nc.vector.tensor_scalar_add(den1[:], uts[hh][64:65, :], 1.0)
                        dens.append(den1)
                        u = npool.tile([64, 512], F32, tag="u", name=f"u{hh}")
                        nc.vector.tensor_copy(u[:], uts[hh][0:64, :])
                        us.append(u)
                    for hh in range(2):
                        po = 64 * hh
                        r = npool.tile([1, 512], F32, tag="r")
                        nc.vector.reciprocal_approx_fast(r[:], dens[hh][:])
                        if tail:
                            # PE ones-matmul broadcast into a free PSUM slot;
                            # shortest-latency path for the final q-block.
                            r16 = npool.tile([1, 512], F16, tag="r16")
                            nc.scalar.copy(r16[:], r[:])
                            rb_ps = pout.tile([64, 512], F32, tag="op", name="rbps")
                            nc.tensor.matmul(rb_ps[:], on_sb[:, 0:64], r16[:],
                                             start=True, stop=True)
                            nc.vector.tensor_mul(
                                at_sb[po:po + 64, p, q0:q0 + 512],
                                us[hh][:], rb_ps[:])
                        else:
                            rb = npool.tile([64, 512], F32, tag="rb")
                            nc.gpsimd.partition_broadcast(rb[:], r[:])
                            nc.vector.tensor_mul(
                                at_sb[po:po + 64, p, q0:q0 + 512], us[hh][:], rb[:])

                # qq-major quarter order: a q-block's two pairs complete in
                # consecutive quarters, so its out-projection spreads over the
                # following quarters instead of bunching at the end.
                QUARTERS = [(p, qq) for qq in range(4) for p in range(2)]
                sched = [(p, qq, i) for (p, qq) in QUARTERS for i in range(KT16)]
                quarters = {}
                hist = []   # per group: [p, qq, i, sc, ex]
                outq = []   # pending out-projection halves
                # scores run one group ahead of exp, and the V-accumulation
                # two behind, so the next ACT's input is always ready the
                # moment the previous ACT retires.
                for it in range(len(sched) + 2):
                    if it < len(sched):
                        p, qq, i = sched[it]
                        q0 = qq * 512
                        sc = psc.tile([128, 1024], F32, tag="sc")
                        for hh in range(2):
                            nc.tensor.matmul(
                                sc[:, hh * 512:(hh + 1) * 512],
                                kt_sb[64 * hh:64 * hh + 64, p, i * 128:(i + 1) * 128],
                                qt_sb[64 * hh:64 * hh + 64, p, q0:q0 + 512],
                                start=True, stop=True,
                            )
                        hist.append([p, qq, i, sc, None])
                    if 1 <= it <= len(sched):
                        e = hist[it - 1]
                        ex = expp.tile([128, 1024], F16, tag="ex")
                        nc.scalar.activation(out=ex[:], in_=e[3][:], func=EXP)
                        e[4] = ex
                        if it == 1:
                            vproj_one(0)   # k-tiles 0,1 behind the first exp
                            vproj_one(1)
                    if it >= 2:
                        g = it - 2
                        pp, pqq, pi, _, pex = hist[g]
                        qi = g // KT16
                        if pi == 0:
                            quarters[(pp, pqq)] = (
                                put.tile([65, 512], F32, tag="ut", name="ut0"),
                                put.tile([65, 512], F32, tag="ut", name="ut1"),
                            )
                        uts = quarters[(pp, pqq)]
                        for hh in range(2):
                            h = 2 * pp + hh
                            nc.tensor.matmul(
                                uts[hh][:],
                                v_sb[:, pi, h * VW:(h + 1) * VW],
                                pex[:, hh * 512:(hh + 1) * 512],
                                start=(pi == 0), stop=(pi == KT16 - 1),
                            )
                        hist[g][4] = None
                        # interleaved extras: vproj stays 2 tiles ahead in the
                        # first quarter; out-proj drains 1 half per group but
                        # only from group 4 on, so the PE never head-of-line
                        # blocks on the previous quarter's normalize muls.
                        if qi == 0:
                            if pi < KT16 - 2:
                                vproj_one(pi + 2)
                        elif outq and pi >= 8:
                            outproj_half(*outq.pop(0))
                        if pi == KT16 - 1:
                            if qi == 0:
                                pv_ctx.__exit__(None, None, None)
                                pout_ctx = tc.tile_pool(name="pout", bufs=2,
                                                        space="PSUM")
                                pout = pout_ctx.__enter__()
                            normalize(uts, pp, pqq * 512,
                                      tail=(g == len(sched) - 1))
                            del quarters[(pp, pqq)]
                            if pp == 1:
                                # q-block pqq fully normalized
                                outq.extend([(pqq * 4 + tt, n)
                                             for tt in range(4) for n in range(2)])
                # final q-block's out-projection (ACT is idle by now);
                # alternate the PSUM->SBUF casts between scalar and vector.
                # The scores/U pools are dead now — recycle their banks into
                # a wide tail pool so all 8 halves pipeline without waiting
                # on cast->slot recycling.  A few LDWEIGHTS keep the PE clock
                # warm through the normalize-chain idle gap.
                pout_ctx.__exit__(None, None, None)
                put_ctx.__exit__(None, None, None)
                psc_ctx.__exit__(None, None, None)
                ptail_ctx = tc.tile_pool(name="ptail", bufs=6, space="PSUM")
                pout = ptail_ctx.__enter__()
                for _ in range(36):
                    nc.tensor.ldweights(warm_sb[:, 0:128])
                for idx, (t, n) in enumerate(outq):
                    outproj_half(t, n, act_copy=(idx % 2 == 0))
                ptail_ctx.__exit__(None, None, None)

    nc.finalize()
    return nc


def kernel(query, key, value, Wq, bq, Wk, bk, Wv, bv, Wo, bo):
    global LAST_RESULT
    if "nc" not in _CACHE:
        _CACHE["nc"] = _build()
    nc = _CACHE["nc"]

    query = np.asarray(query, np.float32)
    key = np.asarray(key, np.float32)
    value = np.asarray(value, np.float32)
    Wq = np.asarray(Wq, np.float32)
    Wk = np.asarray(Wk, np.float32)
    Wv = np.asarray(Wv, np.float32)
    Wo = np.asarray(Wo, np.float32)
    bq = np.asarray(bq, np.float32)
    bk = np.asarray(bk, np.float32)
    bv = np.asarray(bv, np.float32)
    bo = np.asarray(bo, np.float32)

    xqT = [np.ascontiguousarray(query[b].T).astype(np.float16) for b in range(B)]
    xkT = [np.ascontiguousarray(key[b].T).astype(np.float16) for b in range(B)]
    xvT = [np.ascontiguousarray(value[b].T).astype(np.float16) for b in range(B)]

    ones1 = np.ones((1, 128), np.float16)
    in_maps = []
    for c in range(8):
        b, hg = c // 4, c % 4
        r0 = hg * CD
        wq_s = np.ascontiguousarray((Wq[r0:r0 + CD, :] * SCALE).T).astype(np.float16)
        wk_s = np.ascontiguousarray(Wk[r0:r0 + CD, :].T).astype(np.float16)
        wo_s = np.ascontiguousarray(Wo[:, r0:r0 + CD].T).astype(np.float16)
        bq_s = np.ascontiguousarray((bq[r0:r0 + CD] * SCALE).reshape(2, 128).T)  # [128,2]
        bk_s = np.ascontiguousarray(bk[r0:r0 + CD].reshape(2, 128).T)
        # V weights/bias in 260-layout: [64 cols of head | bias-1 col] x4
        wv260 = np.zeros((DM, W260), np.float32)
        bv260 = np.zeros((1, W260), np.float32)
        for hh in range(HLOC):
            wv260[:, hh * VW:hh * VW + HD] = Wv[r0 + hh * HD:r0 + (hh + 1) * HD, :].T
            bv260[0, hh * VW:hh * VW + HD] = bv[r0 + hh * HD:r0 + (hh + 1) * HD]
            bv260[0, hh * VW + HD] = 1.0
        in_maps.append({
            "xq": xqT[b], "xk": xkT[b], "xv": xvT[b],
            "wq": wq_s, "wk": wk_s, "wv": np.ascontiguousarray(wv260).astype(np.float16),
            "wo": wo_s, "bq": bq_s, "bk": bk_s, "bv": bv260.astype(np.float16),
            "ones1": ones1,
        })

    res = run_bass_kernel_spmd(nc, in_maps, core_ids=list(range(8)))
    LAST_RESULT = res

    out = np.empty((B, S, DM), np.float32)
    for b in range(B):
        acc = np.zeros((S, DM), np.float64)
        for hg in range(4):
            acc += res.results[b * 4 + hg]["out"].astype(np.float64)
        out[b] = (acc + bo.astype(np.float64)).astype(np.float32)
    return out


# revision 67
# speedup vs baseline: 1.0192x; 1.0146x over previous
"""Multi-head attention (softmax+1) for TRN2, 8 NeuronCores.

Sharding: data-parallel over batch B=2 (4 cores per batch) x tensor-parallel
over the 16 heads (4 heads per core).  Each core computes its 4 heads'
QKV projections, attention, and a partial output projection; the host sums
the 4 partials per batch and adds the output bias.

Per-core kernel (S=2048, DM=1024, HD=64, Hloc=4):
  QT[d,q] / KT[d,k] head-transposed layouts from x^T inputs (PE matmuls),
  V'[k, 4*65] natural layout with a ones column per head (denominator trick),
  scores^T[k,q] -> exp on ACT (scale folded into Wq) -> U^T = V'^T @ expT
  (row 64 of each head's block = softmax denominator), normalization via
  1/(1+den) broadcast, partial out-projection.

Schedule notes (this revision):
  - Projection bias-adds are emitted immediately after each PSUM
    accumulation stops and alternate DVE/ACT, so qt/kt column blocks
    release early and K's m-loop is not paced by a single engine.
  - PE warmup: ~14 dummy matmuls + LDWEIGHTS gap-fillers keep the HAM
    clock gate at 8/8 through the DMA-gated projection start; a tiny exp
    preloads the ACT table and a dummy partition_broadcast preloads the
    GpSimd Q7 library (otherwise a ~7us swap hits the first boundary).
  - Quarters run qq-major ((0,0),(1,0),(0,1),(1,1),...) so each q-block's
    out-projection becomes available two quarters later and drains evenly
    (1 half per group from group 8) without head-of-line blocking the PE
    behind the previous quarter's normalize muls.
  - The attention phase is ACT(exp)-bound (~1.01us per [128,1024] exp)
    outside the first quarter; the PE stream (scores pair + V-accum +
    interleaved vproj/outproj) fits underneath.
  - Tail: only the last q-block's 8 out-proj halves remain after the last
    exp.  Its normalize splits across ACT (den-adds, r16 casts) and DVE
    (u-copies, recips) with a PE ones-matmul broadcast; the dead scores/U
    PSUM pools are recycled into a wide tail pool and the final casts
    alternate scalar/vector engines.
"""

import contextlib
import sys

if "/opt/trn_rl_repo" not in sys.path:
    sys.path.insert(0, "/opt/trn_rl_repo")

import numpy as np

import concourse.bass as bass
import concourse.mybir as mybir
import concourse.tile as tile
from concourse import bacc
from concourse.bass_utils import run_bass_kernel_spmd

F32 = mybir.dt.float32
F32R = mybir.dt.float32r
F16 = mybir.dt.float16
EXP = mybir.ActivationFunctionType.Exp
IDENT = mybir.ActivationFunctionType.Identity

B, S, DM = 2, 2048, 1024
H, HD = 16, 64
SCALE = HD ** -0.5
HLOC = 4              # heads per core
CD = HLOC * HD        # 256 local head dims
VW = HD + 1           # 65: V columns + ones column per head
MC = DM // 128        # 8 contraction chunks for projections
KT16 = S // 128       # 16 sequence tiles
W260 = HLOC * VW      # 260

_CACHE = {}
LAST_RESULT = None


def _build():
    nc = bacc.Bacc()
    dp = nc.declare_dram_parameter
    xq_d = dp("xq", [DM, S], F16, isOutput=False)    # query[b]^T
    xk_d = dp("xk", [DM, S], F16, isOutput=False)
    xv_d = dp("xv", [DM, S], F16, isOutput=False)
    wq_d = dp("wq", [DM, CD], F16, isOutput=False)   # (SCALE * Wq_shard)^T
    wk_d = dp("wk", [DM, CD], F16, isOutput=False)   # Wk_shard^T
    wv_d = dp("wv", [DM, W260], F16, isOutput=False)  # Wv^T 260-layout, zeros in ones-cols
    wo_d = dp("wo", [CD, DM], F16, isOutput=False)   # Wo_shard^T
    bq_d = dp("bq", [128, 2], F32, isOutput=False)   # bias cols per 128-pair (SCALE-folded)
    bk_d = dp("bk", [128, 2], F32, isOutput=False)
    bv_d = dp("bv", [1, W260], F16, isOutput=False)  # [bv_h | 1.0] blocks
    on_d = dp("ones1", [1, 128], F16, isOutput=False)
    out_d = dp("out", [S, DM], F16, isOutput=True)   # partial (pre-bo) projection

    with tile.TileContext(nc) as tc:
        with tc.tile_pool(name="weights", bufs=1) as wpool, \
             tc.tile_pool(name="persist", bufs=1) as perst:
            wq_sb = wpool.tile([128, MC, CD], F16)
            wk_sb = wpool.tile([128, MC, CD], F16)
            wv_sb = wpool.tile([128, MC, W260], F16)
            wo_sb = wpool.tile([128, 2, DM], F16)
            bq_sb = wpool.tile([128, 2], F32)
            bk_sb = wpool.tile([128, 2], F32)
            bv_sb = wpool.tile([1, W260], F16)
            on_sb = wpool.tile([1, 128], F16)

            qt_sb = perst.tile([128, 2, S], F16)   # [d(2 heads), pair, q]
            kt_sb = perst.tile([128, 2, S], F16)
            v_sb = perst.tile([128, KT16, W260], F16)  # [k, ktile, 4*(V|1)]
            at_sb = perst.tile([128, 2, S], F16)   # normalized attn out^T
            xv_sb = perst.tile([128, MC, S], F16)  # resident value^T chunks
            warm_sb = wpool.tile([128, 512], F16)  # zero tile for PE warmup
            wout_sb = wpool.tile([1, 128], F16)    # scratch for ACT table preload
            ones32 = wpool.tile([1, 64], F32)      # f32 ones row for PE broadcast

            # ------------- Phase 1: Q and K projections ----------------
            # Bias-adds are emitted right after each j-block's accumulation
            # stops and round-robin across DVE/ACT/GpSimd so qt/kt release
            # quickly and the K projection isn't paced by a single engine.
            with tc.tile_pool(name="xs", bufs=16) as xs, \
                 tc.tile_pool(name="pproj", bufs=8, space="PSUM") as pproj:
                # PE warmup: ~20 dummy matmuls on a memset tile lift the HAM
                # clock gate to 8/8 before the real projections start, and a
                # tiny exp preloads the ACT function table off-critical-path.
                # A dummy partition_broadcast makes GpSimd load its Q7 library
                # now instead of at the first quarter boundary (~7us swap).
                nc.vector.memset(warm_sb[:], 0.0)
                nc.vector.memset(ones32[:], 1.0)
                nc.scalar.activation(out=wout_sb[:], in_=warm_sb[0:1, 0:128],
                                     func=EXP)
                # dummy broadcast so GpSimd loads its Q7 library now instead
                # of at the first quarter boundary (~7us swap)
                pbwarm = wpool.tile([2, 64], F32)
                nc.gpsimd.partition_broadcast(pbwarm[:], ones32[:])
                wps = pproj.tile([128, 256], F32, tag="ps", name="warmps")
                for _ in range(14):
                    nc.tensor.matmul(wps[:], warm_sb[:, 0:128], warm_sb[:, 0:256],
                                     start=True, stop=True)

                def pe_keepwarm(n):
                    # dependency-free LDWEIGHTS that fill PE idle slots during
                    # DMA-gated stretches so HAM never drops to 4/8.  No PSUM
                    # write, and every real matmul self-loads its own weights,
                    # so clobbering the weight buffer is harmless.
                    for _ in range(n):
                        nc.tensor.ldweights(warm_sb[:, 0:128])
                nc.sync.dma_start(
                    out=wq_sb[:], in_=wq_d.ap().rearrange("(m p) c -> p m c", m=MC))
                nc.sync.dma_start(out=bq_sb[:], in_=bq_d.ap())

                def bias_add(idx, dst_ap, ps_ap, b_ap):
                    # alternate DVE / ACT so neither engine paces the release
                    # (GpSimd cannot read PSUM)
                    if idx % 2 == 0:
                        nc.vector.tensor_scalar_add(dst_ap, ps_ap, b_ap)
                    else:
                        nc.scalar.activation(out=dst_ap, in_=ps_ap, func=IDENT,
                                             bias=b_ap)

                for src_d, w_sb, b_sb, dst in (
                    (xq_d, wq_sb, bq_sb, qt_sb),
                    (xk_d, wk_sb, bk_sb, kt_sb),
                ):
                    pss = [pproj.tile([128, 512], F32, tag="ps", name=f"ps{k}")
                           for k in range(8)]
                    xts = []
                    for m in range(MC):
                        xt = xs.tile([128, S], F16, tag="xs", name=f"xt{m}")
                        nc.sync.dma_start(out=xt[:], in_=src_d.ap()[m * 128:(m + 1) * 128, :])
                        xts.append(xt)
                    if dst is qt_sb:
                        # K weights enqueue after the xq chunks so xq streams first
                        nc.sync.dma_start(
                            out=wk_sb[:],
                            in_=wk_d.ap().rearrange("(m p) c -> p m c", m=MC))
                        nc.sync.dma_start(out=bk_sb[:], in_=bk_d.ap())
                    for m in range(MC):
                        xt = xts[m]
                        st, sp = (m == 0), (m == MC - 1)
                        for p in range(2):
                            for j in range(4):
                                nc.tensor.matmul(
                                    pss[p * 4 + j][:],
                                    w_sb[:, m, p * 128:(p + 1) * 128],
                                    xt[:, j * 512:(j + 1) * 512],
                                    start=st, stop=sp,
                                )
                                if sp:
                                    # release this 512-col block immediately
                                    bias_add(
                                        p * 4 + j,
                                        dst[:, p, j * 512:(j + 1) * 512],
                                        pss[p * 4 + j][:], b_sb[:, p:p + 1],
                                    )
                        if m < 5:
                            pe_keepwarm(6)
                # stage V weights/input + wo for the attention phase.
                # xv is sent in column quarters so vproj of early k-tiles
                # unblocks after 1MB instead of the full 4MB transfer.
                nc.sync.dma_start(
                    out=wv_sb[:], in_=wv_d.ap().rearrange("(m p) c -> p m c", m=MC))
                nc.sync.dma_start(out=bv_sb[:], in_=bv_d.ap())
                nc.sync.dma_start(out=on_sb[:], in_=on_d.ap())
                for q4 in range(4):
                    c0, c1 = q4 * 512, (q4 + 1) * 512
                    for m in range(MC):
                        nc.sync.dma_start(out=xv_sb[:, m, c0:c1],
                                          in_=xv_d.ap()[m * 128:(m + 1) * 128, c0:c1])
                    if q4 == 1:
                        nc.sync.dma_start(
                            out=wo_sb[:],
                            in_=wo_d.ap().rearrange("(k p) c -> p k c", k=2))

            # ------------- Phase 2: attention, software-pipelined -----------
            with tc.tile_pool(name="expp", bufs=8) as expp, \
                 tc.tile_pool(name="obuf", bufs=4) as obuf, \
                 tc.tile_pool(name="npool", bufs=3) as npool:
                psc_ctx = tc.tile_pool(name="psc", bufs=2, space="PSUM")
                psc = psc_ctx.__enter__()
                put_ctx = tc.tile_pool(name="put", bufs=2, space="PSUM")
                put = put_ctx.__enter__()

                pout = None
                pv_ctx = tc.tile_pool(name="pv", bufs=2, space="PSUM")
                pv = pv_ctx.__enter__()

                def vproj_one(kt):
                    """V projection for one k-tile."""
                    vps = pv.tile([128, W260], F32, tag="vps", name="vps")
                    nc.tensor.matmul(vps[:], on_sb[:], bv_sb[:], start=True, stop=False)
                    for m in range(MC):
                        nc.tensor.matmul(
                            vps[:],
                            xv_sb[:, m, kt * 128:(kt + 1) * 128],
                            wv_sb[:, m, :],
                            start=False, stop=(m == MC - 1),
                        )
                    nc.vector.tensor_copy(v_sb[:, kt, :], vps[:])

                ob_tiles = {}

                def outproj_half(t, n, act_copy=False, pin=True):
                    # Pin the model-time so the scheduler doesn't hoist these
                    # ahead of the normalize muls that produce `at` (its DVE/
                    # GpSimd timing model is optimistic and the PE stream is
                    # in-order, so a hoisted LDWEIGHTS head-of-line blocks).
                    # stagger the pins ~1.2us apart so the 8 halves of a
                    # q-block spread across the drain quarter instead of
                    # bunching right after a shared pin time (which made the
                    # drain groups run ~290ns over the exp budget for 8
                    # consecutive groups)
                    ctx = (tc.tile_wait_until(0.090 + 0.028 * (t // 4)
                                              + 0.0012 * (2 * (t % 4) + n))
                           if pin else contextlib.nullcontext())
                    with ctx:
                        ob = ob_tiles.get(t)
                        if ob is None:
                            ob = obuf.tile([128, DM], F16, tag="ob", name="ob")
                            ob_tiles[t] = ob
                        op = pout.tile([128, 512], F32, tag="op", name="op")
                        for cc in range(2):
                            nc.tensor.matmul(
                                op[:],
                                at_sb[:, cc, t * 128:(t + 1) * 128],
                                wo_sb[:, cc, n * 512:(n + 1) * 512],
                                start=(cc == 0), stop=(cc == 1),
                            )
                        if act_copy:
                            nc.scalar.copy(ob[:, n * 512:(n + 1) * 512], op[:])
                        else:
                            nc.vector.tensor_copy(ob[:, n * 512:(n + 1) * 512], op[:])
                        if n == 1:
                            nc.sync.dma_start(
                                out=out_d.ap()[t * 128:(t + 1) * 128, :], in_=ob[:])
                            del ob_tiles[t]

                def normalize(uts, p, q0, tail=False):
                    # The U accumulator (PSUM) is freed by the den-add (row
                    # 64) + u-copy (rows 0:64); the 1/(1+den) broadcast is a
                    # float32r ones-matmul on the PE (~0.3us, vs ~1us+queue on
                    # GpSimd), so `at` is ready ~2.5us after the quarter ends
                    # and the hoisted out-projection never stalls on it.
                    dens, us = [], []
                    for hh in range(2):
                        den1 = npool.tile([1, 512], F32, tag="den", name=f"den{hh}")
                        if tail:
                            # ACT is idle in the tail: run the den-adds there
                            # so DVE can start the u-copies in parallel.
                            nc.scalar.activation(out=den1[:], in_=uts[hh][64:65, :],
                                                 func=IDENT, bias=1.0)
                        else:
                            nc.vector.tensor_scalar_add(den1[:], uts[hh][64:65, :], 1.0)
                        dens.append(den1)
                        u = npool.tile([64, 512], F32, tag="u", name=f"u{hh}")
                        if hh == 0 and not tail:
                            # split the two U-copies across ACT and DVE so
                            # both ut PSUM slots free ~0.7us sooner at quarter
                            # boundaries (the release chain was DVE-serial)
                            nc.scalar.copy(u[:], uts[hh][0:64, :])
                        else:
                            nc.vector.tensor_copy(u[:], uts[hh][0:64, :])
                        us.append(u)
                    for hh in range(2):
                        po = 64 * hh
                        r = npool.tile([1, 512], F32, tag="r")
                        nc.vector.reciprocal_approx_fast(r[:], dens[hh][:])
                        if tail:
                            # PE ones-matmul broadcast into a free PSUM slot;
                            # shortest-latency path for the final q-block.
                            r16 = npool.tile([1, 512], F16, tag="r16")
                            nc.scalar.copy(r16[:], r[:])
                            rb_ps = pout.tile([64, 512], F32, tag="op", name="rbps")
                            nc.tensor.matmul(rb_ps[:], on_sb[:, 0:64], r16[:],
                                             start=True, stop=True)
                            nc.vector.tensor_mul(
                                at_sb[po:po + 64, p, q0:q0 + 512],
                                us[hh][:], rb_ps[:])
                        else:
                            rb = npool.tile([64, 512], F32, tag="rb")
                            nc.gpsimd.partition_broadcast(rb[:], r[:])
                            nc.vector.tensor_mul(
                                at_sb[po:po + 64, p, q0:q0 + 512], us[hh][:], rb[:])

                # qq-major quarter order: a q-block's two pairs complete in
                # consecutive quarters, so its out-projection spreads over the
                # following quarters instead of bunching at the end.
                QUARTERS = [(p, qq) for qq in range(4) for p in range(2)]
                sched = [(p, qq, i) for (p, qq) in QUARTERS for i in range(KT16)]
                quarters = {}
                hist = []   # per group: [p, qq, i, sc, ex]
                outq = []   # pending out-projection halves
                # scores run one group ahead of exp, and the V-accumulation
                # two behind, so the next ACT's input is always ready the
                # moment the previous ACT retires.
                for it in range(len(sched) + 2):
                    if it < len(sched):
                        p, qq, i = sched[it]
                        q0 = qq * 512
                        sc = psc.tile([128, 1024], F32, tag="sc")
                        for hh in range(2):
                            nc.tensor.matmul(
                                sc[:, hh * 512:(hh + 1) * 512],
                                kt_sb[64 * hh:64 * hh + 64, p, i * 128:(i + 1) * 128],
                                qt_sb[64 * hh:64 * hh + 64, p, q0:q0 + 512],
                                start=True, stop=True,
                            )
                        hist.append([p, qq, i, sc, None])
                    if 1 <= it <= len(sched):
                        e = hist[it - 1]
                        ex = expp.tile([128, 1024], F16, tag="ex")
                        nc.scalar.activation(out=ex[:], in_=e[3][:], func=EXP)
                        e[4] = ex
                        if it == 1:
                            vproj_one(0)   # k-tiles 0,1 behind the first exp
                            vproj_one(1)
                    if it >= 2:
                        g = it - 2
                        pp, pqq, pi, _, pex = hist[g]
                        qi = g // KT16
                        if pi == 0:
                            quarters[(pp, pqq)] = (
                                put.tile([65, 512], F32, tag="ut", name="ut0"),
                                put.tile([65, 512], F32, tag="ut", name="ut1"),
                            )
                        uts = quarters[(pp, pqq)]
                        for hh in range(2):
                            h = 2 * pp + hh
                            nc.tensor.matmul(
                                uts[hh][:],
                                v_sb[:, pi, h * VW:(h + 1) * VW],
                                pex[:, hh * 512:(hh + 1) * 512],
                                start=(pi == 0), stop=(pi == KT16 - 1),
                            )
                        hist[g][4] = None
                        # interleaved extras: vproj stays 2 tiles ahead in the
                        # first quarter; out-proj drains 1 half per group but
                        # only from group 4 on, so the PE never head-of-line
                        # blocks on the previous quarter's normalize muls.
                        if qi == 0:
                            if pi < KT16 - 2:
                                vproj_one(pi + 2)
                        elif outq and pi >= 8:
                            outproj_half(*outq.pop(0))
                        if pi == KT16 - 1:
                            if qi == 0:
                                pv_ctx.__exit__(None, None, None)
                                pout_ctx = tc.tile_pool(name="pout", bufs=2,
                                                        space="PSUM")
                                pout = pout_ctx.__enter__()
                            normalize(uts, pp, pqq * 512,
                                      tail=(g == len(sched) - 1))
                            del quarters[(pp, pqq)]
                            if pp == 1:
                                # q-block pqq fully normalized
                                outq.extend([(pqq * 4 + tt, n)
                                             for tt in range(4) for n in range(2)])
                # final q-block's out-projection (ACT is idle by now);
                # alternate the PSUM->SBUF casts between scalar and vector.
                # The scores/U pools are dead now — recycle their banks into
                # a wide tail pool so all 8 halves pipeline without waiting
                # on cast->slot recycling.  A few LDWEIGHTS keep the PE clock
                # warm through the normalize-chain idle gap.
                pout_ctx.__exit__(None, None, None)
                put_ctx.__exit__(None, None, None)
                psc_ctx.__exit__(None, None, None)
                ptail_ctx = tc.tile_pool(name="ptail", bufs=6, space="PSUM")
                pout = ptail_ctx.__enter__()
                for _ in range(36):
                    nc.tensor.ldweights(warm_sb[:, 0:128])
                for idx, (t, n) in enumerate(outq):
                    outproj_half(t, n, act_copy=(idx % 2 == 0))
                ptail_ctx.__exit__(None, None, None)

    nc.finalize()
    return nc


def kernel(query, key, value, Wq, bq, Wk, bk, Wv, bv, Wo, bo):
    global LAST_RESULT
    if "nc" not in _CACHE:
        _CACHE["nc"] = _build()
    nc = _CACHE["nc"]

    query = np.asarray(query, np.float32)
    key = np.asarray(key, np.float32)
    value = np.asarray(value, np.float32)
    Wq = np.asarray(Wq, np.float32)
    Wk = np.asarray(Wk, np.float32)
    Wv = np.asarray(Wv, np.float32)
    Wo = np.asarray(Wo, np.float32)
    bq = np.asarray(bq, np.float32)
    bk = np.asarray(bk, np.float32)
    bv = np.asarray(bv, np.float32)
    bo = np.asarray(bo, np.float32)

    xqT = [np.ascontiguousarray(query[b].T).astype(np.float16) for b in range(B)]
    xkT = [np.ascontiguousarray(key[b].T).astype(np.float16) for b in range(B)]
    xvT = [np.ascontiguousarray(value[b].T).astype(np.float16) for b in range(B)]

    ones1 = np.ones((1, 128), np.float16)
    in_maps = []
    for c in range(8):
        b, hg = c // 4, c % 4
        r0 = hg * CD
        wq_s = np.ascontiguousarray((Wq[r0:r0 + CD, :] * SCALE).T).astype(np.float16)
        wk_s = np.ascontiguousarray(Wk[r0:r0 + CD, :].T).astype(np.float16)
        wo_s = np.ascontiguousarray(Wo[:, r0:r0 + CD].T).astype(np.float16)
        bq_s = np.ascontiguousarray((bq[r0:r0 + CD] * SCALE).reshape(2, 128).T)  # [128,2]
        bk_s = np.ascontiguousarray(bk[r0:r0 + CD].reshape(2, 128).T)
        # V weights/bias in 260-layout: [64 cols of head | bias-1 col] x4
        wv260 = np.zeros((DM, W260), np.float32)
        bv260 = np.zeros((1, W260), np.float32)
        for hh in range(HLOC):
            wv260[:, hh * VW:hh * VW + HD] = Wv[r0 + hh * HD:r0 + (hh + 1) * HD, :].T
            bv260[0, hh * VW:hh * VW + HD] = bv[r0 + hh * HD:r0 + (hh + 1) * HD]
            bv260[0, hh * VW + HD] = 1.0
        in_maps.append({
            "xq": xqT[b], "xk": xkT[b], "xv": xvT[b],
            "wq": wq_s, "wk": wk_s, "wv": np.ascontiguousarray(wv260).astype(np.float16),
            "wo": wo_s, "bq": bq_s, "bk": bk_s, "bv": bv260.astype(np.float16),
            "ones1": ones1,
        })

    res = run_bass_kernel_spmd(nc, in_maps, core_ids=list(range(8)))
    LAST_RESULT = res

    out = np.empty((B, S, DM), np.float32)
    for b in range(B):
        acc = np.zeros((S, DM), np.float64)
        for hg in range(4):
            acc += res.results[b * 4 + hg]["out"].astype(np.float64)
        out[b] = (acc + bo.astype(np.float64)).astype(np.float32)
    return out
